# revision 12
# baseline (speedup 1.0000x reference)
"""Trainium2 Bass kernel for nn_Attend: softmax(q@k^T * scale + bias) @ v.

Shapes (full problem):
  q:         [B=2, H=8, S=2048, D=64] fp32
  k, v:      [B=2, S=2048, D=64]      fp32 (shared across heads)
  mask:      [B=2, S=2048] bool       (all ones in practice)
  attn_bias: [B=2, H=8, S=2048, S=2048] fp32
  out:       [B=2, H=8, S=2048, D=64] fp32

Sharding: 16 (b,h) pairs over 8 cores -> 2 heads per core, k/v replicated
per-b (4 cores share each b).

Per-core algorithm (fp16 compute, fp32 PSUM accumulation):
  - kT, qT built once via PE transposes into [128(zero-padded d), S] fp16;
    q pre-scaled by 1/sqrt(D). K padded to 128 so every stationary operand
    is full-height (fast-weight-load path; K=64 stationaries stall LDW).
  - S^T[j, i] per (head, 512-i-chunk, 128-j-tile): matmul(kT_tile, qT_chunk)
    into PSUM fp32; bias added by 4 matmuls using the NATURAL [i', j] bias
    block as the STATIONARY operand and a 128x128 identity as the moving
    operand (out = bias_blk.T @ I) accumulating into the same PSUM bank.
  - Bias streams in 1MB quarter-chunks (DMA then fp32->fp16 DVE cast),
    prefetched one chunk ahead with emission ordered so the in-order DVE
    stream never blocks PE at chunk boundaries.
  - P^T = exp(S^T - 2) via ScalarE, PSUM -> SBUF fp16, 1024 cols per
    instruction (softmax is shift-invariant; -2 keeps exp in fp16 range).
  - out^T[d, i] accumulated over j-tiles: matmul(v_aug, P^T), v_aug carrying
    a ones-column -> row 64 of out^T is the softmax denominator. PV matmuls
    run one j-pair behind the exp; the epilogue of chunk c runs inside
    chunk c+1 (both software-pipelined so PE never waits on ScalarE/DVE).
  - Epilogue: small PE transposes back to [i, d], reciprocal + scale,
    out-DMA on the ScalarE HWDGE ring (keeps it off the bias queue).
"""

import sys

sys.path.insert(0, "/opt/trn_rl_repo")

from contextlib import ExitStack

import numpy as np

B, H, S, D = 2, 8, 2048, 64
NH = 2          # heads per core
N_CORES = 8
IC = S // 512   # i-chunks per head
JT = S // 128   # j-tiles
JP = JT // 2    # j-tile pairs
NCHUNK = NH * IC

_cache = {}


def _build():
    import concourse.bacc as bacc
    import concourse.tile as tile
    from concourse import masks, mybir

    f32 = mybir.dt.float32
    f16 = mybir.dt.float16
    Exp = mybir.ActivationFunctionType.Exp

    nc = bacc.Bacc("TRN2", target_bir_lowering=False, debug=False,
                   num_devices=N_CORES)
    q_ap = nc.dram_tensor("q", [NH, S, D], f32, kind="ExternalInput").ap()
    k_ap = nc.dram_tensor("k", [S, D], f32, kind="ExternalInput").ap()
    v_ap = nc.dram_tensor("v", [S, D], f32, kind="ExternalInput").ap()
    bias_ap = nc.dram_tensor("bias", [NH, S, S], f32, kind="ExternalInput").ap()
    out_ap = nc.dram_tensor("out", [NH, S, D], f32, kind="ExternalOutput").ap()

    with tile.TileContext(nc) as tc, ExitStack() as ctx:
        const_pool = ctx.enter_context(tc.tile_pool(name="const", bufs=1))
        prep_sb = ctx.enter_context(tc.tile_pool(name="prep_sb", bufs=1))
        small_ps = ctx.enter_context(
            tc.tile_pool(name="small_ps", bufs=2, space="PSUM"))
        bias_pool = ctx.enter_context(tc.tile_pool(name="bias", bufs=2))
        st_pool = ctx.enter_context(
            tc.tile_pool(name="st", bufs=2, space="PSUM"))
        pt_pool = ctx.enter_context(tc.tile_pool(name="pt", bufs=3))
        ov_pool = ctx.enter_context(
            tc.tile_pool(name="ov", bufs=1, space="PSUM"))
        epi_sb = ctx.enter_context(tc.tile_pool(name="epi_sb", bufs=2))

        ident = const_pool.tile([128, 128], f16)
        masks.make_identity(nc, ident[:])
        ident32 = const_pool.tile([128, 128], f32)
        masks.make_identity(nc, ident32[:])
        shift = const_pool.tile([128, 1], f32)
        nc.vector.memset(shift[:], -2.0)

        # bias tiles: [128, quarter, s, 512] -- DMA and cast per 1MB quarter
        def dma_bias_quarter(bias_f, idx, qr):
            h, c = divmod(idx, IC)
            bsrc = bias_ap[h, c * 512:(c + 1) * 512,
                           qr * 512:(qr + 1) * 512].rearrange(
                "(s p) j -> p s j", p=128)
            nc.sync.dma_start(bias_f[:, qr], bsrc)

        def new_bias_f(idx):
            return bias_pool.tile([128, 4, 4, 512], f32, tag="biasf",
                                  name=f"bias_f{idx}")

        def new_bias_t(idx):
            return bias_pool.tile([128, 4, 4, 512], f16, tag="biast",
                                  name=f"bias_t{idx}")

        def cast_bias_quarter(bias_t, bias_f, qr):
            nc.vector.tensor_copy(bias_t[:, qr], bias_f[:, qr])

        # bias slice for (s, jt): quarter = jt//4, col = (jt%4)*128
        def bias_slice(bias_t, s, jt):
            qr, jc = divmod(jt, 4)
            return bias_t[:, qr, s, jc * 128:(jc + 1) * 128]

        # ---- prep: kT/qT [128(pad), S] f16 (q scaled), v_aug [128, 16*65]
        kT = const_pool.tile([128, S], f16)
        qT = const_pool.tile([128, NH * S], f16)
        v_aug = const_pool.tile([128, JT * 65], f16)
        nc.vector.memset(kT[64:128, :], 0.0)
        nc.vector.memset(qT[64:128, :], 0.0)
        nc.vector.memset(v_aug[:], 1.0)

        kv_f = prep_sb.tile([128, 2, JT, 64], f32, tag="kv", name="kv_f")
        q_f = prep_sb.tile([128, NH, JT, 64], f32, tag="qf", name="q_f")
        # front DMA order = first-use order: k, q_h0, v, bias0, q_h1
        nc.sync.dma_start(kv_f[:, 0], k_ap.rearrange("(t p) d -> p t d", p=128))
        nc.sync.dma_start(
            q_f[:, 0], q_ap[0].rearrange("(t p) d -> p t d", p=128))
        nc.sync.dma_start(kv_f[:, 1], v_ap.rearrange("(t p) d -> p t d", p=128))
        bias_f_cur = new_bias_f(0)
        for qr in range(4):
            dma_bias_quarter(bias_f_cur, 0, qr)
        for h in range(1, NH):
            nc.sync.dma_start(
                q_f[:, h], q_ap[h].rearrange("(t p) d -> p t d", p=128))

        k16 = prep_sb.tile([128, JT, 64], f16, tag="k16", name="k16")
        q16 = prep_sb.tile([128, NH, JT, 64], f16, tag="q16", name="q16")

        def prep_tensor(src_view, dst, dst_off, scale, f16buf):
            # per-group: cast 4 tiles, transpose them, copy out of PSUM
            for g in range(JT // 4):
                if scale is None:
                    nc.vector.tensor_copy(f16buf[:, g * 4:(g + 1) * 4],
                                          src_view[:, g * 4:(g + 1) * 4])
                else:
                    nc.vector.tensor_scalar_mul(
                        f16buf[:, g * 4:(g + 1) * 4],
                        src_view[:, g * 4:(g + 1) * 4], scale)
                p = small_ps.tile([64, 512], f16, tag="sm",
                                  name=f"tp_{dst_off}_{g}")
                for u in range(4):
                    nc.tensor.matmul(p[:, u * 128:(u + 1) * 128],
                                     f16buf[:, g * 4 + u], ident[:],
                                     is_transpose=True, start=True, stop=True)
                nc.vector.tensor_copy(
                    dst[0:64, dst_off + g * 512: dst_off + (g + 1) * 512],
                    p[:])

        prep_tensor(kv_f[:, 0], kT, 0, None, k16)
        prep_tensor(q_f[:, 0], qT, 0, float(D) ** -0.5, q16[:, 0])
        for jt in range(JT):
            nc.scalar.copy(v_aug[:, jt * 65:jt * 65 + 64], kv_f[:, 1, jt])

        # cast chunk-0 bias quarters
        bias_t_cur = new_bias_t(0)
        for qr in range(4):
            cast_bias_quarter(bias_t_cur, bias_f_cur, qr)

        # ---- main loop; epilogue of chunk c-1 is emitted inside chunk c
        epi_state = None   # (ov, h, c)

        def emit_epilogue(state):
            ov, eh, ec = state
            ovs = epi_sb.tile([65, 512], f32, tag="ovs")
            nc.vector.tensor_copy(ovs[:], ov[:])
            res = epi_sb.tile([128, 4, 64], f32, tag="res")
            for s in range(4):
                tp = small_ps.tile([128, 65], f32, tag="sm")
                nc.tensor.matmul(tp[:], ovs[:, s * 128:(s + 1) * 128],
                                 ident32[:65, :65], is_transpose=True,
                                 start=True, stop=True)
                rec = epi_sb.tile([128, 1], f32, tag="rec")
                nc.vector.reciprocal(rec[:], tp[:, 64:65])
                nc.vector.tensor_scalar_mul(res[:, s, :], tp[:, 0:64], rec[:])
            nc.scalar.dma_start(
                out_ap[eh, ec * 512:(ec + 1) * 512, :].rearrange(
                    "(s p) d -> p s d", p=128), res[:])

        for idx in range(NCHUNK):
            h, c = divmod(idx, IC)
            bias_t = bias_t_cur
            # 1. prefetch DMA for next chunk (sync queue)
            if idx + 1 < NCHUNK:
                bias_f_next = new_bias_f(idx + 1)
                for qr in range(4):
                    dma_bias_quarter(bias_f_next, idx + 1, qr)
            ov = ov_pool.tile([65, 512], f32)
            prev_pt = None
            for p in range(JP):
                st = st_pool.tile([128, 1024], f32)
                for u in range(2):
                    jt = 2 * p + u
                    nc.tensor.matmul(
                        st[:, u * 512:(u + 1) * 512],
                        kT[:, jt * 128:(jt + 1) * 128],
                        qT[:, h * S + c * 512: h * S + (c + 1) * 512],
                        start=True, stop=False, skip_group_check=True)
                    for s in range(4):
                        nc.tensor.matmul(
                            st[:, u * 512 + s * 128: u * 512 + (s + 1) * 128],
                            bias_slice(bias_t, s, jt),
                            ident[:], start=False, stop=(s == 3),
                            skip_group_check=True)
                if prev_pt is not None:
                    for u in range(2):
                        jt = 2 * (p - 1) + u
                        nc.tensor.matmul(
                            ov[:], v_aug[:, jt * 65: jt * 65 + 65],
                            prev_pt[:, u * 512:(u + 1) * 512],
                            start=(jt == 0), stop=False,
                            skip_group_check=True)
                pt = pt_pool.tile([128, 1024], f16)
                nc.scalar.activation(pt[:], st[:], Exp, bias=shift[:])
                prev_pt = pt
                if p == 0:
                    # 2. previous chunk's epilogue (PE ops land after pair 0)
                    if epi_state is not None:
                        emit_epilogue(epi_state)
                    if idx == 1:
                        # deferred prep of head 1 (q_h1 DMA lands by now)
                        prep_tensor(q_f[:, 1], qT, S, float(D) ** -0.5,
                                    q16[:, 1])
                    # 3. cast quarters for next chunk (DVE, after epilogue)
                    if idx + 1 < NCHUNK:
                        bias_t_cur = new_bias_t(idx + 1)
                        for qr in range(4):
                            cast_bias_quarter(bias_t_cur, bias_f_next, qr)
            for u in range(2):
                jt = 2 * (JP - 1) + u
                nc.tensor.matmul(
                    ov[:], v_aug[:, jt * 65: jt * 65 + 65],
                    prev_pt[:, u * 512:(u + 1) * 512],
                    start=False, stop=(u == 1), skip_group_check=True)
            epi_state = (ov, h, c)

        emit_epilogue(epi_state)

    nc.compile()
    return nc


def kernel(q, k, v, mask, attn_bias):
    from concourse.bass_utils import run_bass_kernel_spmd

    q = np.ascontiguousarray(np.asarray(q, dtype=np.float32))
    k = np.ascontiguousarray(np.asarray(k, dtype=np.float32))
    v = np.ascontiguousarray(np.asarray(v, dtype=np.float32))
    mask = np.asarray(mask)
    attn_bias = np.asarray(attn_bias, dtype=np.float32)

    if not mask.all():
        attn_bias = np.where(mask[:, None, None, :], attn_bias,
                             np.float32(-3.0e38)).astype(np.float32)

    if "nc" not in _cache:
        _cache["nc"] = _build()
    nc = _cache["nc"]

    in_maps = []
    for c in range(N_CORES):
        b = c // 4
        h0 = NH * (c % 4)
        in_maps.append({
            "q": np.ascontiguousarray(q[b, h0:h0 + NH]),
            "k": k[b],
            "v": v[b],
            "bias": np.ascontiguousarray(attn_bias[b, h0:h0 + NH]),
        })
    res = run_bass_kernel_spmd(nc, in_maps, core_ids=list(range(N_CORES)))
    out = np.empty((B, H, S, D), dtype=np.float32)
    for c in range(N_CORES):
        b = c // 4
        h0 = NH * (c % 4)
        out[b, h0:h0 + NH] = res.results[c]["out"]
    return out


# revision 13
# speedup vs baseline: 1.0893x; 1.0893x over previous
"""Trainium2 Bass kernel for nn_Attend: softmax(q@k^T * scale + bias) @ v.

Shapes (full problem):
  q:         [B=2, H=8, S=2048, D=64] fp32
  k, v:      [B=2, S=2048, D=64]      fp32 (shared across heads)
  mask:      [B=2, S=2048] bool       (all ones in practice)
  attn_bias: [B=2, H=8, S=2048, S=2048] fp32
  out:       [B=2, H=8, S=2048, D=64] fp32

Sharding: 16 (b,h) pairs over 8 cores -> 2 heads per core, k/v replicated
per-b (4 cores share each b).

Per-core algorithm (fp16 compute, fp32 PSUM accumulation):
  - kT, qT built once via PE transposes into [128(zero-padded d), S] fp16;
    q pre-scaled by 1/sqrt(D). K padded to 128 so every stationary operand
    is full-height (fast-weight-load path; K=64 stationaries stall LDW).
  - S^T[j, i] per (head, 512-i-chunk, 128-j-tile): matmul(kT_tile, qT_chunk)
    into PSUM fp32; bias added by 4 matmuls using the NATURAL [i', j] bias
    block as the STATIONARY operand and a 128x128 identity as the moving
    operand (out = bias_blk.T @ I) accumulating into the same PSUM bank.
  - Bias streams in 1MB quarter-chunks (DMA then fp32->fp16 DVE cast),
    prefetched one chunk ahead with emission ordered so the in-order DVE
    stream never blocks PE at chunk boundaries.
  - P^T = exp(S^T - 2) via ScalarE, PSUM -> SBUF fp16, 1024 cols per
    instruction (softmax is shift-invariant; -2 keeps exp in fp16 range).
  - out^T[d, i] accumulated over j-tiles: matmul(v_aug, P^T), v_aug carrying
    a ones-column -> row 64 of out^T is the softmax denominator. PV matmuls
    run one j-pair behind the exp; the epilogue of chunk c runs inside
    chunk c+1 (both software-pipelined so PE never waits on ScalarE/DVE).
  - Epilogue: small PE transposes back to [i, d], reciprocal + scale,
    out-DMA on the ScalarE HWDGE ring (keeps it off the bias queue).
"""

import sys

sys.path.insert(0, "/opt/trn_rl_repo")

from contextlib import ExitStack

import numpy as np

B, H, S, D = 2, 8, 2048, 64
NH = 2          # heads per core
N_CORES = 8
IC = S // 512   # i-chunks per head
JT = S // 128   # j-tiles
JP = JT // 2    # j-tile pairs
NCHUNK = NH * IC

_cache = {}


def _build():
    import concourse.bacc as bacc
    import concourse.tile as tile
    from concourse import masks, mybir

    f32 = mybir.dt.float32
    f16 = mybir.dt.float16
    Exp = mybir.ActivationFunctionType.Exp

    nc = bacc.Bacc("TRN2", target_bir_lowering=False, debug=False,
                   num_devices=N_CORES)
    q_ap = nc.dram_tensor("q", [NH, S, D], f32, kind="ExternalInput").ap()
    k_ap = nc.dram_tensor("k", [S, D], f32, kind="ExternalInput").ap()
    v_ap = nc.dram_tensor("v", [S, D], f32, kind="ExternalInput").ap()
    bias_ap = nc.dram_tensor("bias", [NH, S, S], f32, kind="ExternalInput").ap()
    out_ap = nc.dram_tensor("out", [NH, S, D], f32, kind="ExternalOutput").ap()

    with tile.TileContext(nc) as tc, ExitStack() as ctx:
        const_pool = ctx.enter_context(tc.tile_pool(name="const", bufs=1))
        prep_sb = ctx.enter_context(tc.tile_pool(name="prep_sb", bufs=1))
        small_ps = ctx.enter_context(
            tc.tile_pool(name="small_ps", bufs=2, space="PSUM"))
        bias_pool = ctx.enter_context(tc.tile_pool(name="bias", bufs=2))
        st_pool = ctx.enter_context(
            tc.tile_pool(name="st", bufs=2, space="PSUM"))
        pt_pool = ctx.enter_context(tc.tile_pool(name="pt", bufs=3))
        ov_pool = ctx.enter_context(
            tc.tile_pool(name="ov", bufs=2, space="PSUM"))
        epi_sb = ctx.enter_context(tc.tile_pool(name="epi_sb", bufs=2))

        ident = const_pool.tile([128, 128], f16)
        masks.make_identity(nc, ident[:])
        ident32 = const_pool.tile([128, 128], f32)
        masks.make_identity(nc, ident32[:])
        shift = const_pool.tile([128, 1], f32)
        nc.vector.memset(shift[:], -2.0)

        # bias tiles: [128, quarter, s, 512] -- DMA and cast per 1MB quarter
        def dma_bias_quarter(bias_f, idx, qr):
            h, c = divmod(idx, IC)
            bsrc = bias_ap[h, c * 512:(c + 1) * 512,
                           qr * 512:(qr + 1) * 512].rearrange(
                "(s p) j -> p s j", p=128)
            nc.sync.dma_start(bias_f[:, qr], bsrc)

        def new_bias_f(idx):
            return bias_pool.tile([128, 4, 4, 512], f32, tag="biasf",
                                  name=f"bias_f{idx}")

        def new_bias_t(idx):
            return bias_pool.tile([128, 4, 4, 512], f16, tag="biast",
                                  name=f"bias_t{idx}")

        def cast_bias_quarter(bias_t, bias_f, qr):
            nc.vector.tensor_copy(bias_t[:, qr], bias_f[:, qr])

        # bias slice for (s, jt): quarter = jt//4, col = (jt%4)*128
        def bias_slice(bias_t, s, jt):
            qr, jc = divmod(jt, 4)
            return bias_t[:, qr, s, jc * 128:(jc + 1) * 128]

        # ---- prep: kT/qT [128(pad), S] f16 (q scaled), v_aug [128, 16*65]
        kT = const_pool.tile([128, S], f16)
        qT = const_pool.tile([128, NH * S], f16)
        v_aug = const_pool.tile([128, JT * 65], f16)
        nc.vector.memset(kT[64:128, :], 0.0)
        nc.vector.memset(qT[64:128, :], 0.0)
        nc.vector.memset(v_aug[:], 1.0)

        kv_f = prep_sb.tile([128, 2, JT, 64], f32, tag="kv", name="kv_f")
        q_f = prep_sb.tile([128, NH, JT, 64], f32, tag="qf", name="q_f")
        # front DMA order = first-use order: k, q_h0, v, bias0, q_h1
        nc.sync.dma_start(kv_f[:, 0], k_ap.rearrange("(t p) d -> p t d", p=128))
        nc.sync.dma_start(
            q_f[:, 0], q_ap[0].rearrange("(t p) d -> p t d", p=128))
        nc.sync.dma_start(kv_f[:, 1], v_ap.rearrange("(t p) d -> p t d", p=128))
        bias_f_cur = new_bias_f(0)
        for qr in range(4):
            dma_bias_quarter(bias_f_cur, 0, qr)
        for h in range(1, NH):
            nc.sync.dma_start(
                q_f[:, h], q_ap[h].rearrange("(t p) d -> p t d", p=128))

        k16 = prep_sb.tile([128, JT, 64], f16, tag="k16", name="k16")
        q16 = prep_sb.tile([128, NH, JT, 64], f16, tag="q16", name="q16")

        def prep_tensor(src_view, dst, dst_off, scale, f16buf):
            # per-group: cast 4 tiles, transpose them, copy out of PSUM
            for g in range(JT // 4):
                if scale is None:
                    nc.vector.tensor_copy(f16buf[:, g * 4:(g + 1) * 4],
                                          src_view[:, g * 4:(g + 1) * 4])
                else:
                    nc.vector.tensor_scalar_mul(
                        f16buf[:, g * 4:(g + 1) * 4],
                        src_view[:, g * 4:(g + 1) * 4], scale)
                p = small_ps.tile([64, 512], f16, tag="sm",
                                  name=f"tp_{dst_off}_{g}")
                for u in range(4):
                    nc.tensor.matmul(p[:, u * 128:(u + 1) * 128],
                                     f16buf[:, g * 4 + u], ident[:],
                                     is_transpose=True, start=True, stop=True)
                nc.vector.tensor_copy(
                    dst[0:64, dst_off + g * 512: dst_off + (g + 1) * 512],
                    p[:])

        prep_tensor(kv_f[:, 0], kT, 0, None, k16)
        prep_tensor(q_f[:, 0], qT, 0, float(D) ** -0.5, q16[:, 0])
        for jt in range(JT):
            nc.scalar.copy(v_aug[:, jt * 65:jt * 65 + 64], kv_f[:, 1, jt])

        # cast chunk-0 bias quarters
        bias_t_cur = new_bias_t(0)
        for qr in range(4):
            cast_bias_quarter(bias_t_cur, bias_f_cur, qr)

        # ---- main loop; epilogue of chunk c-1 is emitted inside chunk c
        epi_state = None   # (ov, h, c)

        def emit_epilogue(state):
            ov, eh, ec = state
            ovs = epi_sb.tile([65, 512], f32, tag="ovs")
            nc.vector.tensor_copy(ovs[:], ov[:])
            res = epi_sb.tile([128, 4, 64], f32, tag="res")
            for s in range(4):
                tp = small_ps.tile([128, 65], f32, tag="sm")
                nc.tensor.matmul(tp[:], ovs[:, s * 128:(s + 1) * 128],
                                 ident32[:65, :65], is_transpose=True,
                                 start=True, stop=True)
                rec = epi_sb.tile([128, 1], f32, tag="rec")
                nc.vector.reciprocal(rec[:], tp[:, 64:65])
                nc.vector.tensor_scalar_mul(res[:, s, :], tp[:, 0:64], rec[:])
            nc.scalar.dma_start(
                out_ap[eh, ec * 512:(ec + 1) * 512, :].rearrange(
                    "(s p) d -> p s d", p=128), res[:])

        for idx in range(NCHUNK):
            h, c = divmod(idx, IC)
            bias_t = bias_t_cur
            # 1. prefetch DMA for next chunk (sync queue)
            if idx + 1 < NCHUNK:
                bias_f_next = new_bias_f(idx + 1)
                for qr in range(4):
                    dma_bias_quarter(bias_f_next, idx + 1, qr)
            ov = ov_pool.tile([65, 512], f32)
            prev_pt = None
            for p in range(JP):
                st = st_pool.tile([128, 1024], f32)
                for u in range(2):
                    jt = 2 * p + u
                    nc.tensor.matmul(
                        st[:, u * 512:(u + 1) * 512],
                        kT[:, jt * 128:(jt + 1) * 128],
                        qT[:, h * S + c * 512: h * S + (c + 1) * 512],
                        start=True, stop=False, skip_group_check=True)
                    for s in range(4):
                        nc.tensor.matmul(
                            st[:, u * 512 + s * 128: u * 512 + (s + 1) * 128],
                            bias_slice(bias_t, s, jt),
                            ident[:], start=False, stop=(s == 3),
                            skip_group_check=True)
                if prev_pt is not None:
                    for u in range(2):
                        jt = 2 * (p - 1) + u
                        nc.tensor.matmul(
                            ov[:], v_aug[:, jt * 65: jt * 65 + 65],
                            prev_pt[:, u * 512:(u + 1) * 512],
                            start=(jt == 0), stop=False,
                            skip_group_check=True)
                pt = pt_pool.tile([128, 1024], f16)
                nc.scalar.activation(pt[:], st[:], Exp, bias=shift[:])
                prev_pt = pt
                if p == 0:
                    # 2. previous chunk's epilogue (PE ops land after pair 0)
                    if epi_state is not None:
                        emit_epilogue(epi_state)
                    if idx == 1:
                        # deferred prep of head 1 (q_h1 DMA lands by now)
                        prep_tensor(q_f[:, 1], qT, S, float(D) ** -0.5,
                                    q16[:, 1])
                    # 3. cast quarters for next chunk (DVE, after epilogue)
                    if idx + 1 < NCHUNK:
                        bias_t_cur = new_bias_t(idx + 1)
                        for qr in range(4):
                            cast_bias_quarter(bias_t_cur, bias_f_next, qr)
            for u in range(2):
                jt = 2 * (JP - 1) + u
                nc.tensor.matmul(
                    ov[:], v_aug[:, jt * 65: jt * 65 + 65],
                    prev_pt[:, u * 512:(u + 1) * 512],
                    start=False, stop=(u == 1), skip_group_check=True)
            epi_state = (ov, h, c)

        emit_epilogue(epi_state)

    nc.compile()
    return nc


def kernel(q, k, v, mask, attn_bias):
    from concourse.bass_utils import run_bass_kernel_spmd

    q = np.ascontiguousarray(np.asarray(q, dtype=np.float32))
    k = np.ascontiguousarray(np.asarray(k, dtype=np.float32))
    v = np.ascontiguousarray(np.asarray(v, dtype=np.float32))
    mask = np.asarray(mask)
    attn_bias = np.asarray(attn_bias, dtype=np.float32)

    if not mask.all():
        attn_bias = np.where(mask[:, None, None, :], attn_bias,
                             np.float32(-3.0e38)).astype(np.float32)

    if "nc" not in _cache:
        _cache["nc"] = _build()
    nc = _cache["nc"]

    in_maps = []
    for c in range(N_CORES):
        b = c // 4
        h0 = NH * (c % 4)
        in_maps.append({
            "q": np.ascontiguousarray(q[b, h0:h0 + NH]),
            "k": k[b],
            "v": v[b],
            "bias": np.ascontiguousarray(attn_bias[b, h0:h0 + NH]),
        })
    res = run_bass_kernel_spmd(nc, in_maps, core_ids=list(range(N_CORES)))
    out = np.empty((B, H, S, D), dtype=np.float32)
    for c in range(N_CORES):
        b = c // 4
        h0 = NH * (c % 4)
        out[b, h0:h0 + NH] = res.results[c]["out"]
    return out
